# revision 1
# baseline (speedup 1.0000x reference)
# Trainium2 Bass kernel for nn_Bert_79817672229402 (DeBERTa-style disentangled
# attention transformer). Batch-parallel over 8 NeuronCores (B=8, one batch
# element per core). All shapes hardcoded per the problem spec.
#
# Device layout strategy (per core, per layer):
#   - residual x: token-major [4x128, 768] f32
#   - h = LN(x) token-major, PE-transposed to hT [768, 512] (channel-major, f32r)
#   - qkT [1536, 512] channel-major (f32r) via WqkT matmuls (+bias via K=1 MM)
#   - value/gate token-major [512, 2304] via WvT matmuls (value -> bf16 v_aug
#     with a ones column per head for the softmax denominator; gate -> gelu bf16)
#   - relative-position scores: expansion tables KPexpT/QPexpT [128,1023] per
#     head-pair (bf16), per-head Qrel/KrelRev [512,1023] bf16, skewed into
#     [k,q]/[q,k] tiles via diagonal SBUF->SBUF DMAs, accumulated into the
#     scores PSUM with identity/transpose matmuls (bf16) alongside the f32r
#     q.k term. Softmax over k (= partitions) without max-subtraction:
#     exp on ACT (scale=1/sqrt(3*64), per-partition mask bias), denominator
#     via the ones column of v_aug in the ctx matmul.
#   - ctx token-major per (head, qtile) with fused GLU (sigmoid(l_skip) *
#     gelu(value) + ctx) * gelu-gate, then LN, XBAR-transposed blocks feed the
#     Wo matmul (bf16), residual-added into x.
import math
import os

import numpy as np

S, B, H, NH, I, L, V, BK, MP = 512, 8, 768, 12, 2304, 4, 16384, 32, 512
DH = H // NH          # 64
DV = I // NH          # 192
EPS = 1e-7
SCALE = 1.0 / math.sqrt(3 * DH)
NT = S // 128         # 4 token tiles
NCH = H // 128        # 6 channel tiles
NCI = I // 128        # 18 ctx channel tiles
W = 2 * S - 1         # 1023 expansion width
NJ = 2 * BK - 1       # 63 relative buckets

LAST_RESULT = [None]


def _build_program(nc, mybir, bass, tile, make_identity, heads=NH, layers=L):
    f32 = mybir.dt.float32
    f32r = mybir.dt.float32r
    bf16 = mybir.dt.bfloat16
    AF = mybir.ActivationFunctionType

    # ---------------- DRAM I/O ----------------
    d_x0 = nc.dram_tensor("x0", [S, H], f32, kind="ExternalInput")
    d_mb = nc.dram_tensor("maskbias", [128, NT], f32, kind="ExternalInput")
    d_rel = nc.dram_tensor("rel_emb", [NJ, H], f32, kind="ExternalInput")
    d_relw = nc.dram_tensor("rel_w", [NJ, H], f32, kind="ExternalInput")
    d_relb = nc.dram_tensor("rel_b", [NJ, H], f32, kind="ExternalInput")
    d_s2 = nc.dram_tensor("s2", [NJ, W], bf16, kind="ExternalInput")
    d_s3 = nc.dram_tensor("s3", [NJ, W], bf16, kind="ExternalInput")
    d_wqk = nc.dram_tensor("wqkT", [L, H + 1, 2 * H], f32r, kind="ExternalInput")
    d_wv = nc.dram_tensor("wvT", [L, H, 2 * I], f32r, kind="ExternalInput")
    d_wo = nc.dram_tensor("woT", [L, I, H], bf16, kind="ExternalInput")
    d_sig = nc.dram_tensor("sig", [L, 128, I], bf16, kind="ExternalInput")
    d_ones = nc.dram_tensor("ones", [1, 512], f32r, kind="ExternalInput")
    d_ident = nc.dram_tensor("ident", [128, 128], f32r, kind="ExternalInput")
    d_out = nc.dram_tensor("out", [S, H], f32, kind="ExternalOutput")

    from contextlib import ExitStack

    tc = tile.TileContext(nc)

    with tc, ExitStack() as es:
        pools = {}

        def pool(name, bufs, space="SBUF"):
            if name not in pools:
                pools[name] = es.enter_context(
                    tc.tile_pool(name=name, bufs=bufs, space=space))
            return pools[name]

        const = pool("const", 1)
        xp = pool("xp", 1)
        hp_pool = pool("hp", 2)
        htp = pool("htp", 1)
        qkp = pool("qkp", 1)
        qkbp = pool("qkbp", 2)
        vgp = pool("vgp", 1)
        glup = pool("glup", 1)
        expp = pool("expp", 1)
        qrelp = pool("qrelp", 3)
        krelp = pool("krelp", 3)
        skewp = pool("skewp", 5)
        probp = pool("probp", 5)
        wstream = pool("wstream", 3)
        wstream2 = pool("wstream2", 2)
        sigp = pool("sigp", 1)
        small = pool("small", 3)
        tmpp = pool("tmpp", 4)
        xbarp = pool("xbarp", 4)
        posp = pool("posp", 1)
        # PSUM: 8 banks total. big(3) + sc(2) + ctx(2) + tr(1) = 8
        ps_big = pool("ps_big", 3, space="PSUM")
        ps_sc = pool("ps_sc", 2, space="PSUM")
        ps_ctx = pool("ps_ctx", 2, space="PSUM")
        ps_tr = pool("ps_tr", 1, space="PSUM")

        # ---------------- constants ----------------
        ident_bf = const.tile([128, 128], bf16)
        make_identity(nc, ident_bf)
        ident_fr = const.tile([128, 128], f32r)
        nc.sync.dma_start(ident_fr, d_ident[:])
        ones_tok = const.tile([1, 512], f32r)
        nc.sync.dma_start(ones_tok, d_ones[:])
        ones_j = ones_tok[:, 0:64]
        mb_sb = const.tile([128, NT], f32)
        nc.sync.dma_start(mb_sb, d_mb[:])
        s2_sb = const.tile([NJ, W], bf16)
        nc.sync.dma_start(s2_sb, d_s2[:])
        s3_sb = const.tile([NJ, W], bf16)
        nc.sync.dma_start(s3_sb, d_s3[:])

        # ---------------- LN helper (token-major) ----------------
        def ln_token(x_ap, out_ap, P, D, out_dtype_tag):
            nsub = D // 256
            stats = tmpp.tile([128, nsub, 6], f32, tag="ln_stats", name="ln_stats")
            for i in range(nsub):
                nc.vector.bn_stats(stats[:P, i, :], x_ap[:, i * 256:(i + 1) * 256])
            mv = tmpp.tile([128, 2], f32, tag="ln_mv", name="ln_mv")
            nc.vector.bn_aggr(mv[:P], stats[:P])
            eps_t = tmpp.tile([128, 1], f32, tag="ln_eps", name="ln_eps")
            nc.vector.memset(eps_t[:P], EPS)
            rstd = tmpp.tile([128, 1], f32, tag="ln_rstd", name="ln_rstd")
            nc.scalar.activation(rstd[:P], mv[:P, 1:2], AF.Sqrt, bias=eps_t[:P], scale=1.0)
            nc.vector.reciprocal(rstd[:P], rstd[:P])
            negmr = tmpp.tile([128, 1], f32, tag="ln_negmr", name="ln_negmr")
            nc.vector.tensor_mul(negmr[:P], mv[:P, 0:1], rstd[:P])
            nc.vector.tensor_scalar_mul(negmr[:P], negmr[:P], -1.0)
            nc.scalar.activation(out_ap, x_ap, AF.Identity, bias=negmr[:P], scale=rstd[:P])

        # ---------------- initial x = LN(word_emb[ids]) ----------------
        x_tiles = []
        for t in range(NT):
            xt = xp.tile([128, H], f32, tag=f"x{t}", name=f"x{t}")
            x_tiles.append(xt)
            x0t = tmpp.tile([128, H], f32, tag="x0stage", name="x0stage", bufs=2)
            nc.sync.dma_start(x0t, d_x0[t * 128:(t + 1) * 128, :])
            ln_token(x0t[:], xt[:], 128, H, f32)

        # ---------------- rel path: rel_ln = LN(rel_emb)*w + b, relT ----------------
        relt_stage = const.tile([NJ, H], f32)
        nc.sync.dma_start(relt_stage, d_rel[:])
        ln_token(relt_stage[:], relt_stage[:], NJ, H, f32)
        relw_t = tmpp.tile([NJ, H], f32, tag="relw", name="relw", bufs=1)
        nc.sync.dma_start(relw_t, d_relw[:])
        nc.vector.tensor_mul(relt_stage[:], relt_stage[:], relw_t[:])
        relb_t = tmpp.tile([NJ, H], f32, tag="relb", name="relb", bufs=1)
        nc.sync.dma_start(relb_t, d_relb[:])
        rel_fin = const.tile([NJ, H], f32r)
        nc.vector.tensor_add(rel_fin[:], relt_stage[:], relb_t[:])
        # transpose -> relT [128, NCH, 64] f32r (col 63 zero via sliced identity)
        relT = const.tile([128, NCH, 64], f32r)
        for c in range(NCH):
            pt = ps_tr.tile([128, 128], f32r, tag="tr_ps", name="tr_ps")
            nc.tensor.transpose(pt[:, :64], rel_fin[:, c * 128:(c + 1) * 128],
                                ident_fr[:NJ, :64])
            nc.vector.tensor_copy(relT[:, c, :], pt[:, :64])

        # ================ layers ================
        for l in range(layers):
            # ---- h = LN(x), hT via PE transpose ----
            hT = [htp.tile([128, 512], f32r, tag=f"hT{c}", name=f"hT{c}") for c in range(NCH)]
            for t in range(NT):
                ht = hp_pool.tile([128, H], f32r, tag="h", name="h")
                ln_token(x_tiles[t][:], ht[:], 128, H, f32r)
                for c in range(NCH):
                    pt = ps_tr.tile([128, 128], f32r, tag="tr_ps", name="tr_ps")
                    nc.tensor.transpose(pt[:], ht[:, c * 128:(c + 1) * 128], ident_fr)
                    nc.vector.tensor_copy(hT[c][:, t * 128:(t + 1) * 128], pt[:])

            # ---- qkT [12 m-tiles][128, 512] f32r ----
            qkT = []
            for m in range(2 * H // 128):
                psq = ps_big.tile([128, 512], f32, tag="big", name="big")
                for c in range(NCH):
                    wchunk = wstream.tile([128, 128], f32r, tag="wqk_l", name="wqk_l")
                    nc.sync.dma_start(wchunk, d_wqk[l, c * 128:(c + 1) * 128, m * 128:(m + 1) * 128])
                    nc.tensor.matmul(psq, wchunk, hT[c][:], start=(c == 0), stop=False)
                brow = wstream.tile([1, 128], f32r, tag="wqk_b", name="wqk_b")
                nc.sync.dma_start(brow, d_wqk[l, H:H + 1, m * 128:(m + 1) * 128])
                nc.tensor.matmul(psq, brow, ones_tok, start=False, stop=True,
                                 skip_group_check=True)
                qt = qkp.tile([128, 512], f32r, tag=f"qkT{m}", name=f"qkT{m}")
                nc.vector.tensor_copy(qt[:], psq)
                qkT.append(qt)

            # ---- value (token-major, bf16 v_aug with ones cols) + gate ----
            v_aug = [vgp.tile([128, NH, DV + 1], bf16, tag=f"vaug{t}", name=f"vaug{t}") for t in range(NT)]
            gate = [vgp.tile([128, I], bf16, tag=f"gate{t}", name=f"gate{t}") for t in range(NT)]
            for t in range(NT):
                nc.vector.memset(v_aug[t][:, :, DV:DV + 1], 1.0)
            NCHUNK = 384
            for t in range(NT):
                for n in range(2 * I // NCHUNK):   # 12 chunks: 6 value + 6 gate
                    psv = ps_big.tile([128, NCHUNK], f32, tag="big", name="big")
                    for c in range(NCH):
                        wchunk = wstream2.tile([128, NCHUNK], f32r, tag="wv_l", name="wv_l")
                        nc.sync.dma_start(
                            wchunk, d_wv[l, c * 128:(c + 1) * 128, n * NCHUNK:(n + 1) * NCHUNK])
                        nc.tensor.matmul(psv, hT[c][:, t * 128:(t + 1) * 128], wchunk,
                                         start=(c == 0), stop=(c == NCH - 1))
                    if n < 6:  # value: two head slices of 192
                        for hh in range(2):
                            h_idx = n * 2 + hh
                            nc.vector.tensor_copy(
                                v_aug[t][:, h_idx, 0:DV], psv[:, hh * DV:(hh + 1) * DV])
                    else:      # gate: gelu -> bf16
                        gn = n - 6
                        nc.scalar.activation(
                            gate[t][:, gn * NCHUNK:(gn + 1) * NCHUNK], psv,
                            AF.Gelu, bias=0.0, scale=1.0)

            # ---- pos projection [63, 1536] bf16 (j-major) ----
            pos_sb = posp.tile([NJ, 2 * H], bf16, tag="pos", name="pos")
            for n in range(3):
                psp = ps_big.tile([128, 512], f32, tag="big", name="big")
                for c in range(NCH):
                    wchunk = wstream2.tile([128, 512], f32r, tag="wqk_pos", name="wqk_pos")
                    nc.sync.dma_start(
                        wchunk, d_wqk[l, c * 128:(c + 1) * 128, n * 512:(n + 1) * 512])
                    nc.tensor.matmul(psp[:64], relT[:, c, :], wchunk,
                                     start=(c == 0), stop=False)
                brow = wstream.tile([1, 512], f32r, tag="wqk_posb", name="wqk_posb")
                nc.sync.dma_start(brow, d_wqk[l, H:H + 1, n * 512:(n + 1) * 512])
                nc.tensor.matmul(psp[:64], ones_j, brow, start=False, stop=True,
                                 skip_group_check=True)
                nc.vector.tensor_copy(pos_sb[:, n * 512:(n + 1) * 512], psp[:NJ])

            # ---- sigmoid(l_skip) replicated, bf16 ----
            sig_sb = sigp.tile([128, I], bf16, tag="sig", name="sig")
            nc.sync.dma_start(sig_sb, d_sig[l])

            glu = [glup.tile([128, I], bf16, tag=f"glu{t}", name=f"glu{t}") for t in range(NT)]

            # ---- per head-pair ----
            for hpi in range(heads // 2):
                # expansion tables for this pair: rows = 128 channels (2 heads)
                kpexp = expp.tile([128, W], bf16, tag="kpexp", name="kpexp")
                qpexp = expp.tile([128, W], bf16, tag="qpexp", name="qpexp")
                for (dst, src_off, s_sb) in ((kpexp, H + hpi * 128, s2_sb),
                                             (qpexp, hpi * 128, s3_sb)):
                    pos_bf = tmpp.tile([NJ, 128], bf16, tag="posbf", name="posbf")
                    nc.vector.tensor_copy(pos_bf[:], pos_sb[:, src_off:src_off + 128])
                    for sc in range(2):
                        w0, w1 = sc * 512, min(W, (sc + 1) * 512)
                        pse = ps_big.tile([128, 512], f32, tag="big", name="big")
                        nc.tensor.matmul(pse[:, :w1 - w0], pos_bf[:], s_sb[:, w0:w1],
                                         start=True, stop=True)
                        nc.vector.tensor_copy(dst[:, w0:w1], pse[:, :w1 - w0])

                # bf16 copies of q/k rows for this pair
                q_bf = qkbp.tile([128, 512], bf16, tag="q_bf", name="q_bf")
                nc.scalar.copy(q_bf[:], qkT[hpi][:])
                k_bf = qkbp.tile([128, 512], bf16, tag="k_bf", name="k_bf")
                nc.scalar.copy(k_bf[:], qkT[NCH + hpi][:])

                for hh in range(2):
                    h_idx = hpi * 2 + hh
                    r0 = hh * 64
                    qT_fr = qkT[hpi][r0:r0 + 64, :]          # [64, 512] f32r
                    kT_fr = qkT[NCH + hpi][r0:r0 + 64, :]
                    qT_bf = q_bf[r0:r0 + 64, :]
                    kT_bf = k_bf[r0:r0 + 64, :]
                    kpe = kpexp[r0:r0 + 64, :]               # [64, 1023] bf16
                    qpe = qpexp[r0:r0 + 64, :]

                    # Qrel [512, 1023] bf16 (rows q) -> skewed term2 [q,k] per q-tile
                    term2 = []
                    for qt in range(NT):
                        qr = qrelp.tile([128, W], bf16, tag="qrel", name="qrel")
                        for sc in range(2):
                            w0, w1 = sc * 512, min(W, (sc + 1) * 512)
                            psr = ps_big.tile([128, 512], f32, tag="big", name="big")
                            nc.tensor.matmul(psr[:, :w1 - w0],
                                             qT_bf[:, qt * 128:(qt + 1) * 128],
                                             kpe[:, w0:w1], start=True, stop=True)
                            nc.vector.tensor_copy(qr[:, w0:w1], psr[:, :w1 - w0])
                        t2 = skewp.tile([128, 512], bf16, tag="term2", name="term2")
                        src = bass.AP(qr.tensor, qr.offset + (S - 1 - qt * 128),
                                      [[W - 1, 128], [1, 512]])
                        nc.sync.dma_start(t2[:], src)
                        term2.append(t2)
                    # KrelRev [512, 1023] bf16 (rows k) -> skewed term3 [k,q] per k-tile
                    term3 = []
                    for kt in range(NT):
                        kr = krelp.tile([128, W], bf16, tag="krel", name="krel")
                        for sc in range(2):
                            w0, w1 = sc * 512, min(W, (sc + 1) * 512)
                            psr = ps_big.tile([128, 512], f32, tag="big", name="big")
                            nc.tensor.matmul(psr[:, :w1 - w0],
                                             kT_bf[:, kt * 128:(kt + 1) * 128],
                                             qpe[:, w0:w1], start=True, stop=True)
                            nc.scalar.copy(kr[:, w0:w1], psr[:, :w1 - w0])
                        t3 = skewp.tile([128, 512], bf16, tag="term3", name="term3")
                        src = bass.AP(kr.tensor, kr.offset + (S - 1 - kt * 128),
                                      [[W - 1, 128], [1, 512]])
                        nc.sync.dma_start(t3[:], src)
                        term3.append(t3)

                    # scoresT PSUM per k-tile; exp -> probsT bf16
                    probsT = []
                    for kt in range(NT):
                        pss = ps_sc.tile([128, 512], f32, tag="sc_ps", name="sc_ps")
                        nc.tensor.matmul(pss, kT_fr[:, kt * 128:(kt + 1) * 128], qT_fr,
                                         start=True, stop=True)
                        nc.tensor.matmul(pss, ident_bf, term3[kt][:],
                                         start=False, stop=True, skip_group_check=True)
                        for qt in range(NT):
                            nc.tensor.matmul(pss[:, qt * 128:(qt + 1) * 128],
                                             term2[qt][:, kt * 128:(kt + 1) * 128],
                                             ident_bf, start=False, stop=True,
                                             skip_group_check=True)
                        pb = probp.tile([128, 512], bf16, tag="probsT", name="probsT")
                        nc.scalar.activation(pb[:], pss, AF.Exp,
                                             bias=mb_sb[:, kt:kt + 1], scale=SCALE)
                        probsT.append(pb)

                    # ctx per q-tile + fused GLU slice
                    for qt in range(NT):
                        psc = ps_ctx.tile([128, DV + 1], f32, tag="ctx_ps", name="ctx_ps")
                        for kt in range(NT):
                            nc.tensor.matmul(psc, probsT[kt][:, qt * 128:(qt + 1) * 128],
                                             v_aug[kt][:, h_idx, :],
                                             start=(kt == 0), stop=(kt == NT - 1))
                        recip = small.tile([128, 1], f32, tag="recip", name="recip")
                        nc.vector.reciprocal(recip, psc[:, DV:DV + 1])
                        cslice = small.tile([128, DV], f32, tag="cslice", name="cslice")
                        nc.vector.tensor_scalar_mul(cslice[:], psc[:, 0:DV], recip)
                        # GLU: glu = (ctx + sig*gelu(v)) * gate
                        vs = small.tile([128, DV], f32, tag="vskip", name="vskip")
                        nc.scalar.activation(vs[:], v_aug[qt][:, h_idx, 0:DV],
                                             AF.Gelu, bias=0.0, scale=1.0)
                        nc.vector.tensor_mul(vs[:], vs[:], sig_sb[:, h_idx * DV:(h_idx + 1) * DV])
                        nc.vector.tensor_add(cslice[:], cslice[:], vs[:])
                        nc.vector.tensor_mul(glu[qt][:, h_idx * DV:(h_idx + 1) * DV],
                                             cslice[:], gate[qt][:, h_idx * DV:(h_idx + 1) * DV])

            # ---- LN(glu) in-place -> ctxln bf16; XBAR blocks feed Wo; residual add ----
            ctxln = glu
            for t in range(NT):
                ln_token(glu[t][:], glu[t][:], 128, I, bf16)
            for t in range(NT):
                psA = ps_big.tile([128, 384], f32, tag="big", name="big")
                psB = ps_big.tile([128, 384], f32, tag="big", name="big")
                for ct in range(NCI):
                    xb = xbarp.tile([128, 128], bf16, tag="xbar", name="xbar")
                    nc.sync.dma_start(xb[:], ctxln[t][:, ct * 128:(ct + 1) * 128],
                                      transpose=True)
                    woA = wstream2.tile([128, 384], bf16, tag="wo_l", name="wo_l")
                    nc.sync.dma_start(woA, d_wo[l, ct * 128:(ct + 1) * 128, 0:384])
                    nc.tensor.matmul(psA, xb[:], woA, start=(ct == 0), stop=(ct == NCI - 1))
                    woB = wstream2.tile([128, 384], bf16, tag="wo_l2", name="wo_l2")
                    nc.sync.dma_start(woB, d_wo[l, ct * 128:(ct + 1) * 128, 384:768])
                    nc.tensor.matmul(psB, xb[:], woB, start=(ct == 0), stop=(ct == NCI - 1))
                nc.vector.tensor_add(x_tiles[t][:, 0:384], x_tiles[t][:, 0:384], psA)
                nc.vector.tensor_add(x_tiles[t][:, 384:768], x_tiles[t][:, 384:768], psB)

        # ---------------- output ----------------
        for t in range(NT):
            nc.sync.dma_start(d_out[t * 128:(t + 1) * 128, :], x_tiles[t][:])

    return nc


def _prepare(inputs):
    os.environ.setdefault("JAX_PLATFORMS", "cpu")
    import ml_dtypes
    import concourse.bass as bass
    import concourse.tile as tile
    import concourse.mybir as mybir
    from concourse import bacc
    from concourse.bass_utils import run_bass_kernel_spmd
    from concourse.masks import make_identity

    ids = np.asarray(inputs["input_ids"])            # [S, B] int32
    amask = np.asarray(inputs["attention_mask"])     # [B,1,1,S] bool
    pidx = np.asarray(inputs["position_indices"])    # [S, S] int32 in [0,62]
    word_emb = np.asarray(inputs["word_emb"], np.float32)
    rel_emb = np.asarray(inputs["rel_emb"], np.float32)
    rel_w = np.asarray(inputs["rel_ln_w"], np.float32)
    rel_b = np.asarray(inputs["rel_ln_b"], np.float32)
    Wv = np.asarray(inputs["Wv"], np.float32)        # [L, 2I, H]
    Wqk = np.asarray(inputs["Wqk"], np.float32)      # [L, 2H, H]
    bqk = np.asarray(inputs["bqk"], np.float32)      # [L, 2H]
    Wo = np.asarray(inputs["Wo"], np.float32)        # [L, H, I]
    l_skip = np.asarray(inputs["l_skip"], np.float32)  # [L, I]

    # ---- host prep ----
    # Toeplitz diagonal table T[s] = idx[q, q + s - 511]
    T = np.zeros(W, np.int64)
    for s in range(W):
        r = s - 511
        q0 = max(0, -r)
        T[s] = pidx[q0, q0 + r]
    T = np.clip(T, 0, NJ - 1)
    S2 = np.zeros((NJ, W), np.float32)
    S2[T, np.arange(W)] = 1.0                         # col s -> one-hot T[s]
    S3 = np.zeros((NJ, W), np.float32)
    S3[T[::-1], np.arange(W)] = 1.0                   # col s' -> one-hot T[1022-s']

    wqkT = np.concatenate([np.transpose(Wqk, (0, 2, 1)),
                           bqk[:, None, :]], axis=1).copy()   # [L, 769, 1536]
    wvT = np.transpose(Wv, (0, 2, 1)).copy()                  # [L, 768, 4608]
    woT = np.transpose(Wo, (0, 2, 1)).astype(ml_dtypes.bfloat16)  # [L, 2304, 768]
    sig = 1.0 / (1.0 + np.exp(-l_skip))                       # [L, I]
    sig_rep = np.broadcast_to(sig[:, None, :], (L, 128, I)).astype(ml_dtypes.bfloat16).copy()
    s2b = S2.astype(ml_dtypes.bfloat16)
    s3b = S3.astype(ml_dtypes.bfloat16)
    relw_rep = np.broadcast_to(rel_w[None, :], (NJ, H)).astype(np.float32).copy()
    relb_rep = np.broadcast_to(rel_b[None, :], (NJ, H)).astype(np.float32).copy()

    nc = bacc.Bacc("TRN2", target_bir_lowering=False)
    _build_program(nc, mybir, bass, tile, make_identity)
    nc.compile()

    in_maps = []
    for b in range(B):
        x0 = word_emb[ids[:, b]].astype(np.float32)           # [S, H]
        mb = (-1e30 * amask[b, 0, 0, :].astype(np.float32))   # [S]
        mb_cols = mb.reshape(NT, 128).T.copy()                # [128, NT]
        in_maps.append({
            "x0": x0, "maskbias": mb_cols, "ones": np.ones((1, 512), np.float32), "ident": np.eye(128, dtype=np.float32),
            "rel_emb": rel_emb, "rel_w": relw_rep, "rel_b": relb_rep,
            "s2": s2b, "s3": s3b,
            "wqkT": wqkT, "wvT": wvT, "woT": woT, "sig": sig_rep,
        })

    return nc, in_maps


def kernel(**inputs):
    from concourse.bass_utils import run_bass_kernel_spmd
    nc, in_maps = _prepare(inputs)
    res = run_bass_kernel_spmd(nc, in_maps, core_ids=list(range(B)))
    LAST_RESULT[0] = res
    out = np.stack([r["out"] for r in res.results], axis=1)   # [S, B, H]
    return out.astype(np.float32)


def bench(inputs, iters=8):
    """Build once, execute repeatedly with device-resident inputs.
    Returns (min_wall_seconds_per_exec, full_output [S,B,H])."""
    import time as _time
    import jax
    from jax.experimental.shard_map import shard_map
    from jax.sharding import Mesh, PartitionSpec, NamedSharding
    import concourse.mybir as mybir
    from concourse import bass2jax

    nc, in_maps = _prepare(inputs)
    bass2jax.install_neuronx_cc_hook()

    partition_name = nc.partition_id_tensor.name if nc.partition_id_tensor else None
    in_names, out_names, out_avals, zero_outs = [], [], [], []
    for alloc in nc.m.functions[0].allocations:
        if not isinstance(alloc, mybir.MemoryLocationSet):
            continue
        name = alloc.memorylocations[0].name
        if alloc.kind == "ExternalInput":
            if name != partition_name:
                in_names.append(name)
        elif alloc.kind == "ExternalOutput":
            shape = tuple(alloc.tensor_shape)
            dtype = mybir.dt.np(alloc.dtype)
            out_names.append(name)
            out_avals.append(jax.core.ShapedArray(shape, dtype))
            zero_outs.append(np.zeros(shape, dtype))
    n_params = len(in_names)
    n_outs = len(out_avals)
    all_in_names = list(in_names) + list(out_names)
    if partition_name is not None:
        all_in_names.append(partition_name)

    def _body(*args):
        operands = list(args)
        if partition_name is not None:
            operands.append(bass2jax.partition_id_tensor())
        outs = bass2jax._bass_exec_p.bind(
            *operands,
            out_avals=tuple(out_avals),
            in_names=tuple(all_in_names),
            out_names=tuple(out_names),
            lowering_input_output_aliases=(),
            sim_require_finite=True,
            sim_require_nnan=True,
            nc=nc,
        )
        return tuple(outs)

    devices = jax.devices()[:B]
    mesh = Mesh(np.asarray(devices), ("core",))
    P_ = PartitionSpec("core")
    sharded = jax.jit(
        shard_map(_body, mesh=mesh, in_specs=(P_,) * (n_params + n_outs),
                  out_specs=(P_,) * n_outs, check_rep=False),
        keep_unused=True)
    concat_in = [np.concatenate([np.asarray(in_maps[c][nm]) for c in range(B)], axis=0)
                 for nm in in_names]
    concat_zeros = [np.zeros((B * z.shape[0], *z.shape[1:]), z.dtype) for z in zero_outs]
    sh = NamedSharding(mesh, P_)
    dev_in = [jax.device_put(a, sh) for a in concat_in]
    dev_zero = [jax.device_put(a, sh) for a in concat_zeros]
    outs = sharded(*dev_in, *dev_zero)
    jax.block_until_ready(outs)
    times = []
    for _ in range(iters):
        t0 = _time.perf_counter()
        o = sharded(*dev_in, *dev_zero)
        jax.block_until_ready(o)
        times.append(_time.perf_counter() - t0)
    oi = out_names.index("out")
    full = np.asarray(outs[oi]).reshape(B, S, H).transpose(1, 0, 2)
    return min(times), full.astype(np.float32), times

